# revision 11
# baseline (speedup 1.0000x reference)
"""Causal single-head attention (B=4, S=2048, D=1024) on 8 trn2 NeuronCores.

Sharding: core = (batch b, parity h).  Each core owns the 1024 queries of
batch b in 256-row blocks {2t+h : t=0..3} (interleaved for causal load
balance) and projects Q/K/V for those same 1024 rows only; K and V are
pair all-gathered (one combined collective per 512-row stage).

All device inputs are pre-cast to bf16 on the host (the device matmuls
are bf16 either way, so numerics are identical to casting on-chip).

Per core (SPMD-uniform):
  phase 0: K/V own-row projections in dt-major 256-row half-stages so the
           first matmul only needs one wk chunk + one xq chunk (early PE
           start); combined K+V pair-all-gather per 512-row stage; then Q.
  phase 1: per slot-pair p (q-blocks 2p,2p+1 local; 512 queries):
       scoresT[k,q] = KT^T QT   k-tiles 0..nsh-1 at N=512 + 4 extra
                                k-tiles at N=256 (later slot only)
       expT = exp(scoresT/32) * mask   (multiplicative 0/1 causal mask)
       out[q,e] += expT^T V            (exp stationary -> no transposes)
       den[q,1] += expT^T ones         (ones-column matmul, own psum bank)
       out_sb = po * recip(den)        (ScalarE evac + DVE recip, bf16)
"""

import os
import sys
from contextlib import ExitStack

import numpy as np
import ml_dtypes

import concourse.bass as bass
import concourse.mybir as mybir
import concourse.tile as tile
from concourse import bacc
from concourse import bass_utils

B, S, D = 4, 2048, 1024
P = 128
QB = 256          # queries per slot
NSLOT = 4         # slots per core
NQ = QB * NSLOT   # queries per core
NCORES = 8
F32 = mybir.dt.float32
BF16 = mybir.dt.bfloat16
SCALE = 1.0 / 32.0  # 1/sqrt(D)


def _build_kernel():
    nc = bacc.Bacc("TRN2", target_bir_lowering=False, debug=False,
                   num_devices=NCORES)

    xq4 = nc.dram_tensor("xq4", [4, P, 8, 256], BF16, kind="ExternalInput").ap()
    wq2 = nc.dram_tensor("wq2", [P, 8, D], BF16, kind="ExternalInput").ap()
    wk2 = nc.dram_tensor("wk2", [P, 8, D], BF16, kind="ExternalInput").ap()
    wv2 = nc.dram_tensor("wv2", [P, 8, D], BF16, kind="ExternalInput").ap()
    maskT = nc.dram_tensor("maskT", [P, 4, 512], BF16, kind="ExternalInput").ap()
    out = nc.dram_tensor("out", [NQ, D], BF16, kind="ExternalOutput").ap()
    # cores 2b (h=0) and 2b+1 (h=1) of batch b exchange K/V halves
    GROUPS = [[0, 1], [2, 3], [4, 5], [6, 7]]

    with tile.TileContext(nc) as tc, ExitStack() as ctx:
        const = ctx.enter_context(tc.tile_pool(name="const", bufs=1))
        persist = ctx.enter_context(tc.tile_pool(name="persist", bufs=1))

        ones = const.tile([P, 1], BF16)
        nc.gpsimd.memset(ones[:], 1.0)
        mask_sb = const.tile([P, 4, 512], BF16)

        QT = persist.tile([P, 8, NQ], BF16)    # [e_in_tile, e_tile, q]
        KT = persist.tile([P, 8, S], BF16)     # [e_in_tile, e_tile, k]
        V = persist.tile([P, 16, D], BF16)     # [k_in_tile, k_tile, e]
        denT = persist.tile([P, 2 * NSLOT], F32)
        rinv = persist.tile([P, 2 * NSLOT], F32)

        # ---------------- projection + gather phase ----------------
        with tc.tile_pool(name="wsb", bufs=1) as wsb_pool, \
             tc.tile_pool(name="xsb", bufs=1) as xsb_pool, \
             tc.tile_pool(name="kvh", bufs=2) as kvh_pool, \
             tc.tile_pool(name="ccdram", bufs=2, space="DRAM") as ccdram, \
             tc.tile_pool(name="pproj", bufs=8, space="PSUM") as pproj:

            wq_sb = wsb_pool.tile([P, 8, D], BF16, tag="wq")
            wk_sb = wsb_pool.tile([P, 8, D], BF16, tag="wk")
            wv_sb = wsb_pool.tile([P, 8, D], BF16, tag="wv")
            xq_sb = xsb_pool.tile([P, 8, NQ], BF16, tag="xq")

            def load_w_chunk(dst, dram, dt):
                nc.sync.dma_start(dst[:, dt:dt + 1, :], dram[:, dt:dt + 1, :])

            def load_x_chunk(c):
                nc.sync.dma_start(xq_sb[:, :, 256 * c:256 * (c + 1)], xq4[c])

            # loads in consumption order (sync HWDGE queue is FIFO); wk/wv
            # go dt-chunked so the dt-major half-stages start early
            load_w_chunk(wk_sb, wk2, 0)
            load_x_chunk(0)
            for dt in range(1, 8):
                load_w_chunk(wk_sb, wk2, dt)
            load_x_chunk(1)
            for dt in range(8):
                load_w_chunk(wv_sb, wv2, dt)
            load_x_chunk(2)
            load_x_chunk(3)
            nc.sync.dma_start(wq_sb[:], wq2[:])
            nc.sync.dma_start(mask_sb[:], maskT[:])

            def emit_k_half(s, half, kth):
                # KT for 256 own rows (xq cols 512s+256*half ..+256),
                # dt-major: each wk chunk unlocks the next 8 matmuls
                qoff = 512 * s + 256 * half
                pss = [pproj.tile([P, 512], F32, tag="pp",
                                  name=f"ppk{s}{half}{i}")
                       for i in range(8)]
                for dt in range(8):
                    for et in range(8):
                        nc.tensor.matmul(
                            pss[et][:, 0:256],
                            wk_sb[:, dt, P * et:P * (et + 1)],
                            xq_sb[:, dt, qoff:qoff + 256],
                            start=(dt == 0), stop=(dt == 7))
                for et in range(8):
                    nc.vector.tensor_copy(
                        kth[:, et, 256 * half:256 * (half + 1)],
                        pss[et][:, 0:256])

            def emit_v_half(s, half, vh):
                pss = [pproj.tile([P, 512], F32, tag="pp",
                                  name=f"ppv{s}{half}{i}")
                       for i in range(4)]
                for dt in range(8):
                    for k2 in range(2):
                        base = 512 * s + 256 * half + P * k2
                        for ec in range(2):
                            nc.tensor.matmul(
                                pss[2 * k2 + ec][:],
                                xq_sb[:, dt, base:base + P],
                                wv_sb[:, dt, 512 * ec:512 * (ec + 1)],
                                start=(dt == 0), stop=(dt == 7))
                for k2 in range(2):
                    for ec in range(2):
                        nc.vector.tensor_copy(
                            vh[:, 2 * half + k2, 512 * ec:512 * (ec + 1)],
                            pss[2 * k2 + ec][:])

            def emit_gather(s, kth, vh):
                # combined K+V all-gather for stage s (one collective:
                # per-gather fixed cost ~8us dominates split variants).
                # Rank-ordered: rank r piece j = global block 4s+2j+r.
                cc_in = ccdram.tile([P, 8192], BF16, tag="cc_in")
                cc_out = ccdram.tile([2, P, 8192], BF16, tag="cc_out")
                nc.gpsimd.dma_start(cc_in[:, 0:4096],
                                    kth[:].rearrange("p a b -> p (a b)"))
                nc.gpsimd.dma_start(cc_in[:, 4096:8192],
                                    vh[:].rearrange("p a b -> p (a b)"))
                nc.gpsimd.collective_compute(
                    "AllGather", mybir.AluOpType.bypass,
                    replica_groups=GROUPS,
                    ins=[cc_in[:]], outs=[cc_out[:]])
                # scatters ride the scalar HWDGE ring: they must not
                # head-of-line-block the next gather's cc_in DMAs
                # (gpsimd ring) nor the input loads (sync ring)
                for r in range(2):
                    cok = cc_out[r, :, 0:4096].rearrange(
                        "p (a b) -> p a b", a=8)
                    cov = cc_out[r, :, 4096:8192].rearrange(
                        "p (a b) -> p a b", a=4)
                    for j in range(2):
                        g = 4 * s + 2 * j + r
                        nc.scalar.dma_start(
                            KT[:, :, 256 * g:256 * (g + 1)],
                            cok[:, :, 256 * j:256 * (j + 1)])
                        nc.scalar.dma_start(
                            V[:, 2 * g:2 * g + 2, :],
                            cov[:, 2 * j:2 * j + 2, :])

            def emit_q(qc):
                for et in range(8):
                    ps = pproj.tile([P, 512], F32, tag="pp")
                    for dt in range(8):
                        nc.tensor.matmul(
                            ps[:], wq_sb[:, dt, P * et:P * (et + 1)],
                            xq_sb[:, dt, 512 * qc:512 * (qc + 1)],
                            start=(dt == 0), stop=(dt == 7))
                    nc.vector.tensor_copy(
                        QT[:, et, 512 * qc:512 * (qc + 1)], ps[:])

            kth0 = kvh_pool.tile([P, 8, 512], BF16, tag="kth", name="kth0")
            vh0 = kvh_pool.tile([P, 4, D], BF16, tag="vh", name="vh0")
            emit_k_half(0, 0, kth0)
            emit_k_half(0, 1, kth0)
            emit_v_half(0, 0, vh0)
            emit_v_half(0, 1, vh0)
            emit_gather(0, kth0, vh0)
            kth1 = kvh_pool.tile([P, 8, 512], BF16, tag="kth", name="kth1")
            vh1 = kvh_pool.tile([P, 4, D], BF16, tag="vh", name="vh1")
            emit_k_half(1, 0, kth1)
            emit_k_half(1, 1, kth1)
            emit_v_half(1, 0, vh1)
            emit_v_half(1, 1, vh1)
            emit_gather(1, kth1, vh1)
            emit_q(0)
            emit_q(1)

        # ---------------- attention phase ----------------
        # Slot pairs (2p, 2p+1) share k-tiles 0..nsh-1 (nsh = 4(2p+1)) at
        # N=512 covering both slots' queries; the later slot's 4 extra
        # k-tiles run at N=256 on the right half.  PV accumulates in [q,e]
        # orientation (exp stationary); the denominator accumulates as an
        # extra ones-column matmul into its own psum bank.
        with tc.tile_pool(name="ps_s", bufs=3, space="PSUM") as ps_s, \
             tc.tile_pool(name="ps_d", bufs=1, space="PSUM") as ps_d, \
             tc.tile_pool(name="ps_o", bufs=2, space="PSUM") as ps_o, \
             tc.tile_pool(name="expp", bufs=2) as expp, \
             tc.tile_pool(name="tmpp", bufs=2) as tmpp, \
             tc.tile_pool(name="osb", bufs=4) as osbp:

            po_den = ps_d.tile([P, 2 * NSLOT], F32)

            for p in range(2):
                nsh = 4 * (2 * p + 1)      # shared k-tiles
                ntot = nsh + 4             # + later slot's extra k-tiles
                expbuf = expp.tile([P, 16, 512], BF16, tag="expbuf")

                # scores + exp over the shared range at N=512
                for kt in range(nsh):
                    ps = ps_s.tile([P, 512], F32, tag="ps")
                    for et in range(8):
                        nc.tensor.matmul(
                            ps[:], KT[:, et, P * kt:P * (kt + 1)],
                            QT[:, et, 512 * p:512 * (p + 1)],
                            start=(et == 0), stop=(et == 7))
                    j = kt - (nsh - 4)
                    if j >= 0:
                        tmp = tmpp.tile([P, 512], BF16, tag="tmp")
                        nc.scalar.activation(tmp[:], ps[:],
                                             mybir.ActivationFunctionType.Exp,
                                             scale=SCALE)
                        nc.vector.tensor_tensor(expbuf[:, kt, :], tmp[:],
                                                mask_sb[:, j, :],
                                                mybir.AluOpType.mult)
                    else:
                        nc.scalar.activation(expbuf[:, kt, :], ps[:],
                                             mybir.ActivationFunctionType.Exp,
                                             scale=SCALE)

                # later slot's extra k-tiles at N=256 (right half)
                for ex in range(4):
                    kt = nsh + ex
                    ps = ps_s.tile([P, 512], F32, tag="ps")
                    for et in range(8):
                        nc.tensor.matmul(
                            ps[:, 0:256], KT[:, et, P * kt:P * (kt + 1)],
                            QT[:, et, 512 * p + 256:512 * (p + 1)],
                            start=(et == 0), stop=(et == 7))
                    tmp = tmpp.tile([P, 512], BF16, tag="tmp")
                    nc.scalar.activation(tmp[:, 0:256], ps[:, 0:256],
                                         mybir.ActivationFunctionType.Exp,
                                         scale=SCALE)
                    nc.vector.tensor_tensor(expbuf[:, kt, 0:256],
                                            tmp[:, 0:256],
                                            mask_sb[:, ex, 0:256],
                                            mybir.AluOpType.mult)

                # PV in [q,e]: one psum tile per 128-query subtile c.
                # c=0,1 -> slot 2p (k-tiles 0..nsh-1, exp cols 128c+..)
                # c=2,3 -> slot 2p+1 (k-tiles 0..ntot-1; extras use the
                #          exp tile's left half, shared tiles the right)
                for c in range(4):
                    idx = 4 * p + c
                    po = ps_o.tile([P, 2, 512], F32, tag="po")
                    nk = nsh if c < 2 else ntot
                    for kt in range(nk):
                        if kt < nsh:
                            ecol = 128 * c
                        else:
                            ecol = 128 * (c - 2)
                        st, sp = (kt == 0), (kt == nk - 1)
                        for eh in range(2):
                            nc.tensor.matmul(
                                po[:, eh, :],
                                expbuf[:, kt, ecol:ecol + P],
                                V[:, kt, 512 * eh:512 * (eh + 1)],
                                start=st, stop=sp)
                        nc.tensor.matmul(
                            po_den[:, idx:idx + 1],
                            expbuf[:, kt, ecol:ecol + P],
                            ones[:, 0:1],
                            start=st, stop=sp)
                    # ScalarE evacuates den (safe w/ concurrent PE writes
                    # to the same bank; DVE would not be), DVE reciprocal
                    nc.scalar.copy(denT[:, idx:idx + 1],
                                   po_den[:, idx:idx + 1])
                    nc.vector.reciprocal(rinv[:, idx:idx + 1],
                                         denT[:, idx:idx + 1])
                    ob = osbp.tile([P, 2, 512], BF16, tag="ob")
                    for eh in range(2):
                        nc.scalar.mul(ob[:, eh, :], po[:, eh, :],
                                      rinv[:, idx:idx + 1])
                    r0 = P * idx
                    nc.sync.dma_start(
                        out[r0:r0 + P, :],
                        ob[:].rearrange("p a b -> p (a b)"))

    nc.compile()
    return nc


_NC_CACHE = None


def _get_nc():
    global _NC_CACHE
    if _NC_CACHE is None:
        _NC_CACHE = _build_kernel()
    return _NC_CACHE


def _make_masks():
    kk = np.arange(P)[:, None]
    qq = np.arange(256)[None, :]
    diag0 = (qq >= kk).astype(np.float32)
    diag1 = (qq >= kk + P).astype(np.float32)
    m = {}
    for h in range(2):
        mt = np.zeros((P, 4, 512), np.float32)
        mt[:, :, 256:] = 1.0  # right half (the later slot of a pair)
        if h == 0:
            mt[:, 0, :256], mt[:, 1, :256] = diag0, diag1
        else:
            mt[:, 0, :256], mt[:, 1, :256] = 1.0, 1.0
            mt[:, 2, :256], mt[:, 3, :256] = diag0, diag1
        m[h] = mt.astype(ml_dtypes.bfloat16)
    return m


def _prep_inputs(x, Wq, Wk, Wv):
    def w2(W):
        return np.ascontiguousarray(
            np.asarray(W, np.float32).reshape(8, P, D).transpose(1, 0, 2)
        ).astype(ml_dtypes.bfloat16)

    wq2, wk2, wv2 = w2(Wq), w2(Wk), w2(Wv)
    masks = _make_masks()
    in_maps = []
    for core in range(NCORES):
        b, h = divmod(core, 2)
        xb = np.asarray(x[b], np.float32)
        order = np.concatenate(
            [np.arange(QB * (2 * t + h), QB * (2 * t + h) + QB)
             for t in range(NSLOT)])
        xq = xb[order]
        xq4 = np.ascontiguousarray(
            xq.reshape(4, 256, 8, P).transpose(0, 3, 2, 1)
        ).astype(ml_dtypes.bfloat16)
        in_maps.append({
            "xq4": xq4,
            "wq2": wq2, "wk2": wk2, "wv2": wv2,
            "maskT": masks[h],
        })
    return in_maps


def run(inputs, trace=False):
    nc = _get_nc()
    in_maps = _prep_inputs(inputs["x"], inputs["Wq"], inputs["Wk"],
                           inputs["Wv"])
    res = bass_utils.run_bass_kernel_spmd(
        nc, in_maps, core_ids=list(range(NCORES)), trace=trace)
    out = np.empty((B, S, D), np.float32)
    for core in range(NCORES):
        b, h = divmod(core, 2)
        oc = np.asarray(res.results[core]["out"]).astype(np.float32)
        for t in range(NSLOT):
            out[b, QB * (2 * t + h):QB * (2 * t + h) + QB] = \
                oc[QB * t:QB * t + QB]
    return out, res


def kernel(**inputs):
    out, _ = run(inputs, trace=False)
    return out


# revision 16
# speedup vs baseline: 1.1601x; 1.1601x over previous
"""Causal single-head attention (B=4, S=2048, D=1024) on 8 trn2 NeuronCores.

Sharding: core = (batch b, parity h).  Each core owns the 1024 queries of
batch b in 256-row blocks {2t+h : t=0..3} (interleaved for causal load
balance) and projects Q/K/V for those same 1024 rows only; K and V are
pair all-gathered (one combined collective per 512-row stage).

All device inputs are pre-cast to bf16 on the host (the device matmuls
are bf16 either way, so numerics are identical to casting on-chip).

Per core (SPMD-uniform):
  phase 0: K/V own-row projections in dt-major 256-row half-stages so the
           first matmul only needs one wk chunk + one xq chunk (early PE
           start); combined K+V pair-all-gather per 512-row stage; then Q.
  phase 1: per slot-pair p (q-blocks 2p,2p+1 local; 512 queries):
       scoresT[k,q] = KT^T QT   k-tiles 0..nsh-1 at N=512 + 4 extra
                                k-tiles at N=256 (later slot only)
       expT = exp(scoresT/32) * mask   (multiplicative 0/1 causal mask)
       out[q,e] += expT^T V            (exp stationary -> no transposes)
       den[q,1] += expT^T ones         (ones-column matmul, own psum bank)
       out_sb = po * recip(den)        (ScalarE evac + DVE recip, bf16)
"""

import os
import sys
from contextlib import ExitStack

import numpy as np
import ml_dtypes

import concourse.bass as bass
import concourse.mybir as mybir
import concourse.tile as tile
from concourse import bacc
from concourse import bass_utils

B, S, D = 4, 2048, 1024
P = 128
QB = 256          # queries per slot
NSLOT = 4         # slots per core
NQ = QB * NSLOT   # queries per core
NCORES = 8
F32 = mybir.dt.float32
BF16 = mybir.dt.bfloat16
SCALE = 1.0 / 32.0  # 1/sqrt(D)


def _build_kernel():
    nc = bacc.Bacc("TRN2", target_bir_lowering=False, debug=False,
                   num_devices=NCORES)

    xq4 = nc.dram_tensor("xq4", [4, P, 8, 256], BF16, kind="ExternalInput").ap()
    wq2 = nc.dram_tensor("wq2", [P, 8, D], BF16, kind="ExternalInput").ap()
    wk2 = nc.dram_tensor("wk2", [P, 8, D], BF16, kind="ExternalInput").ap()
    wv2 = nc.dram_tensor("wv2", [P, 8, D], BF16, kind="ExternalInput").ap()
    maskT = nc.dram_tensor("maskT", [P, 4, 512], BF16, kind="ExternalInput").ap()
    out = nc.dram_tensor("out", [NQ, D], BF16, kind="ExternalOutput").ap()
    # cores 2b (h=0) and 2b+1 (h=1) of batch b exchange K/V halves
    GROUPS = [[0, 1], [2, 3], [4, 5], [6, 7]]

    with tile.TileContext(nc) as tc, ExitStack() as ctx:
        const = ctx.enter_context(tc.tile_pool(name="const", bufs=1))
        persist = ctx.enter_context(tc.tile_pool(name="persist", bufs=1))

        ones = const.tile([P, 1], BF16)
        nc.gpsimd.memset(ones[:], 1.0)
        mask_sb = const.tile([P, 4, 512], BF16)

        QT = persist.tile([P, 8, NQ], BF16)    # [e_in_tile, e_tile, q]
        KT = persist.tile([P, 8, S], BF16)     # [e_in_tile, e_tile, k]
        V = persist.tile([P, 16, D], BF16)     # [k_in_tile, k_tile, e]
        denT = persist.tile([P, 2 * NSLOT], F32)
        rinv = persist.tile([P, 2 * NSLOT], F32)

        # ---------------- projection + gather phase ----------------
        with tc.tile_pool(name="wsb", bufs=1) as wsb_pool, \
             tc.tile_pool(name="xsb", bufs=1) as xsb_pool, \
             tc.tile_pool(name="kvh", bufs=2) as kvh_pool, \
             tc.tile_pool(name="ccdram", bufs=2, space="DRAM") as ccdram, \
             tc.tile_pool(name="pproj", bufs=8, space="PSUM") as pproj:

            wq_sb = wsb_pool.tile([P, 8, D], BF16, tag="wq")
            wk_sb = wsb_pool.tile([P, 8, D], BF16, tag="wk")
            wv_sb = wsb_pool.tile([P, 8, D], BF16, tag="wv")
            xq_sb = xsb_pool.tile([P, 8, NQ], BF16, tag="xq")

            # PE warm-up: dummy matmuls on a memset tile starting ~4us in,
            # while input DMAs are still in flight.  Flips the HAM clock
            # gate to 8/8 (full 2.4 GHz) before the first real matmul --
            # otherwise the first ~3.4us of projections run at half clock.
            warm = const.tile([P, 512], BF16)
            nc.vector.memset(warm[:], 0.0)
            pwarm = pproj.tile([P, 512], F32, tag="pp", name="pwarm")
            for _ in range(24):
                nc.tensor.matmul(pwarm[:], warm[:, 0:P], warm[:],
                                 start=True, stop=True)

            def load_w_chunk(dst, dram, dt):
                nc.sync.dma_start(dst[:, dt:dt + 1, :], dram[:, dt:dt + 1, :])

            def load_x_chunk(c):
                nc.sync.dma_start(xq_sb[:, :, 256 * c:256 * (c + 1)], xq4[c])

            # loads in consumption order (sync HWDGE queue is FIFO); wk/wv
            # go dt-chunked so the dt-major half-stages start early
            load_w_chunk(wk_sb, wk2, 0)
            load_x_chunk(0)
            for dt in range(1, 8):
                load_w_chunk(wk_sb, wk2, dt)
            load_x_chunk(1)
            for dt in range(8):
                load_w_chunk(wv_sb, wv2, dt)
            load_x_chunk(2)
            load_x_chunk(3)
            nc.sync.dma_start(wq_sb[:], wq2[:])
            nc.sync.dma_start(mask_sb[:], maskT[:])

            def emit_k_half(s, half, kth):
                # KT for 256 own rows (xq cols 512s+256*half ..+256),
                # dt-major: each wk chunk unlocks the next 8 matmuls
                qoff = 512 * s + 256 * half
                pss = [pproj.tile([P, 512], F32, tag="pp",
                                  name=f"ppk{s}{half}{i}")
                       for i in range(8)]
                for dt in range(8):
                    for et in range(8):
                        nc.tensor.matmul(
                            pss[et][:, 0:256],
                            wk_sb[:, dt, P * et:P * (et + 1)],
                            xq_sb[:, dt, qoff:qoff + 256],
                            start=(dt == 0), stop=(dt == 7))
                for et in range(8):
                    nc.vector.tensor_copy(
                        kth[:, et, 256 * half:256 * (half + 1)],
                        pss[et][:, 0:256])

            def emit_v_half(s, half, vh):
                pss = [pproj.tile([P, 512], F32, tag="pp",
                                  name=f"ppv{s}{half}{i}")
                       for i in range(4)]
                for dt in range(8):
                    for k2 in range(2):
                        base = 512 * s + 256 * half + P * k2
                        for ec in range(2):
                            nc.tensor.matmul(
                                pss[2 * k2 + ec][:],
                                xq_sb[:, dt, base:base + P],
                                wv_sb[:, dt, 512 * ec:512 * (ec + 1)],
                                start=(dt == 0), stop=(dt == 7))
                for k2 in range(2):
                    for ec in range(2):
                        nc.vector.tensor_copy(
                            vh[:, 2 * half + k2, 512 * ec:512 * (ec + 1)],
                            pss[2 * k2 + ec][:])

            # Gathers are split (K0, V0, K1, V1: 1MB each) so the first
            # deadline is covered as early as possible, and all cc_in
            # staging DMAs ride the scalar HWDGE ring, emitted BEFORE any
            # scatter, so no trigger is head-of-line-blocked.  The gpsimd
            # ring carries only the collective ops themselves.
            def emit_cc(buf, tag):
                cc_in = ccdram.tile([P, 4096], BF16, tag=tag + "i",
                                    name=tag + "i")
                cc_out = ccdram.tile([2, P, 4096], BF16, tag=tag + "o",
                                     name=tag + "o")
                nc.scalar.dma_start(cc_in[:],
                                    buf[:].rearrange("p a b -> p (a b)"))
                nc.gpsimd.collective_compute(
                    "AllGather", mybir.AluOpType.bypass,
                    replica_groups=GROUPS,
                    ins=[cc_in[:]], outs=[cc_out[:]])
                return cc_out

            def scatter_k(s, cc_out):
                for r in range(2):
                    cok = cc_out[r].rearrange("p (a b) -> p a b", a=8)
                    for j in range(2):
                        g = 4 * s + 2 * j + r
                        nc.scalar.dma_start(
                            KT[:, :, 256 * g:256 * (g + 1)],
                            cok[:, :, 256 * j:256 * (j + 1)])

            def scatter_v(s, cc_out):
                for r in range(2):
                    cov = cc_out[r].rearrange("p (a b) -> p a b", a=4)
                    for j in range(2):
                        g = 4 * s + 2 * j + r
                        nc.scalar.dma_start(
                            V[:, 2 * g:2 * g + 2, :],
                            cov[:, 2 * j:2 * j + 2, :])

            def emit_q(qc):
                for et in range(8):
                    ps = pproj.tile([P, 512], F32, tag="pp")
                    for dt in range(8):
                        nc.tensor.matmul(
                            ps[:], wq_sb[:, dt, P * et:P * (et + 1)],
                            xq_sb[:, dt, 512 * qc:512 * (qc + 1)],
                            start=(dt == 0), stop=(dt == 7))
                    nc.vector.tensor_copy(
                        QT[:, et, 512 * qc:512 * (qc + 1)], ps[:])

            kth0 = kvh_pool.tile([P, 8, 512], BF16, tag="kth", name="kth0")
            vh0 = kvh_pool.tile([P, 4, D], BF16, tag="vh", name="vh0")
            kth1 = kvh_pool.tile([P, 8, 512], BF16, tag="kth", name="kth1")
            vh1 = kvh_pool.tile([P, 4, D], BF16, tag="vh", name="vh1")
            emit_k_half(0, 0, kth0)
            emit_k_half(0, 1, kth0)
            cck0 = emit_cc(kth0, "ck0")
            emit_v_half(0, 0, vh0)
            emit_v_half(0, 1, vh0)
            ccv0 = emit_cc(vh0, "cv0")
            emit_k_half(1, 0, kth1)
            emit_k_half(1, 1, kth1)
            cck1 = emit_cc(kth1, "ck1")
            emit_v_half(1, 0, vh1)
            emit_v_half(1, 1, vh1)
            ccv1 = emit_cc(vh1, "cv1")
            scatter_k(0, cck0)
            scatter_v(0, ccv0)
            scatter_k(1, cck1)
            scatter_v(1, ccv1)
            emit_q(0)
            emit_q(1)

        # ---------------- attention phase ----------------
        # Slot pairs (2p, 2p+1) share k-tiles 0..nsh-1 (nsh = 4(2p+1)) at
        # N=512 covering both slots' queries; the later slot's 4 extra
        # k-tiles run at N=256 on the right half.  PV accumulates in [q,e]
        # orientation (exp stationary); the denominator accumulates as an
        # extra ones-column matmul into its own psum bank.
        with tc.tile_pool(name="ps_s", bufs=3, space="PSUM") as ps_s, \
             tc.tile_pool(name="ps_d", bufs=1, space="PSUM") as ps_d, \
             tc.tile_pool(name="ps_o", bufs=4, space="PSUM") as ps_o, \
             tc.tile_pool(name="expp", bufs=2) as expp, \
             tc.tile_pool(name="tmpp", bufs=2) as tmpp, \
             tc.tile_pool(name="osb", bufs=4) as osbp:

            po_den = ps_d.tile([P, 2 * NSLOT], F32)

            for p in range(2):
                nsh = 4 * (2 * p + 1)      # shared k-tiles
                ntot = nsh + 4             # + later slot's extra k-tiles
                expbuf = expp.tile([P, 16, 512], BF16, tag="expbuf")

                # scores + exp over the shared range at N=512
                for kt in range(nsh):
                    ps = ps_s.tile([P, 512], F32, tag="ps")
                    for et in range(8):
                        nc.tensor.matmul(
                            ps[:], KT[:, et, P * kt:P * (kt + 1)],
                            QT[:, et, 512 * p:512 * (p + 1)],
                            start=(et == 0), stop=(et == 7))
                    j = kt - (nsh - 4)
                    if j >= 0:
                        tmp = tmpp.tile([P, 512], BF16, tag="tmp")
                        nc.scalar.activation(tmp[:], ps[:],
                                             mybir.ActivationFunctionType.Exp,
                                             scale=SCALE)
                        nc.vector.tensor_tensor(expbuf[:, kt, :], tmp[:],
                                                mask_sb[:, j, :],
                                                mybir.AluOpType.mult)
                    else:
                        nc.scalar.activation(expbuf[:, kt, :], ps[:],
                                             mybir.ActivationFunctionType.Exp,
                                             scale=SCALE)

                # later slot's extra k-tiles at N=256 (right half)
                for ex in range(4):
                    kt = nsh + ex
                    ps = ps_s.tile([P, 512], F32, tag="ps")
                    for et in range(8):
                        nc.tensor.matmul(
                            ps[:, 0:256], KT[:, et, P * kt:P * (kt + 1)],
                            QT[:, et, 512 * p + 256:512 * (p + 1)],
                            start=(et == 0), stop=(et == 7))
                    tmp = tmpp.tile([P, 512], BF16, tag="tmp")
                    nc.scalar.activation(tmp[:, 0:256], ps[:, 0:256],
                                         mybir.ActivationFunctionType.Exp,
                                         scale=SCALE)
                    nc.vector.tensor_tensor(expbuf[:, kt, 0:256],
                                            tmp[:, 0:256],
                                            mask_sb[:, ex, 0:256],
                                            mybir.AluOpType.mult)

                # PV in [q,e]: one psum tile per 128-query subtile c.
                # c=0,1 -> slot 2p (k-tiles 0..nsh-1, exp cols 128c+..)
                # c=2,3 -> slot 2p+1 (k-tiles 0..ntot-1; extras use the
                #          exp tile's left half, shared tiles the right)
                for c in range(4):
                    idx = 4 * p + c
                    # one single-bank psum tile per (c, eh) so each half
                    # frees independently after its normalize
                    pos = [ps_o.tile([P, 512], F32, tag="po",
                                     name=f"po{p}{c}{i}") for i in range(2)]
                    nk = nsh if c < 2 else ntot
                    for kt in range(nk):
                        if kt < nsh:
                            ecol = 128 * c
                        else:
                            ecol = 128 * (c - 2)
                        st, sp = (kt == 0), (kt == nk - 1)
                        for eh in range(2):
                            nc.tensor.matmul(
                                pos[eh][:],
                                expbuf[:, kt, ecol:ecol + P],
                                V[:, kt, 512 * eh:512 * (eh + 1)],
                                start=st, stop=sp)
                        nc.tensor.matmul(
                            po_den[:, idx:idx + 1],
                            expbuf[:, kt, ecol:ecol + P],
                            ones[:, 0:1],
                            start=st, stop=sp)
                    # ScalarE evacuates den (safe w/ concurrent PE writes
                    # to the same bank; DVE would not be), DVE reciprocal
                    nc.scalar.copy(denT[:, idx:idx + 1],
                                   po_den[:, idx:idx + 1])
                    nc.vector.reciprocal(rinv[:, idx:idx + 1],
                                         denT[:, idx:idx + 1])
                    r0 = P * idx
                    for eh in range(2):
                        ob = osbp.tile([P, 512], BF16, tag="ob",
                                       name=f"ob{p}{c}{eh}")
                        nc.scalar.mul(ob[:], pos[eh][:],
                                      rinv[:, idx:idx + 1])
                        nc.sync.dma_start(
                            out[r0:r0 + P, 512 * eh:512 * (eh + 1)], ob[:])

    nc.compile()
    return nc


_NC_CACHE = None


def _get_nc():
    global _NC_CACHE
    if _NC_CACHE is None:
        _NC_CACHE = _build_kernel()
    return _NC_CACHE


def _make_masks():
    kk = np.arange(P)[:, None]
    qq = np.arange(256)[None, :]
    diag0 = (qq >= kk).astype(np.float32)
    diag1 = (qq >= kk + P).astype(np.float32)
    m = {}
    for h in range(2):
        mt = np.zeros((P, 4, 512), np.float32)
        mt[:, :, 256:] = 1.0  # right half (the later slot of a pair)
        if h == 0:
            mt[:, 0, :256], mt[:, 1, :256] = diag0, diag1
        else:
            mt[:, 0, :256], mt[:, 1, :256] = 1.0, 1.0
            mt[:, 2, :256], mt[:, 3, :256] = diag0, diag1
        m[h] = mt.astype(ml_dtypes.bfloat16)
    return m


def _prep_inputs(x, Wq, Wk, Wv):
    def w2(W):
        return np.ascontiguousarray(
            np.asarray(W, np.float32).reshape(8, P, D).transpose(1, 0, 2)
        ).astype(ml_dtypes.bfloat16)

    wq2, wk2, wv2 = w2(Wq), w2(Wk), w2(Wv)
    masks = _make_masks()
    in_maps = []
    for core in range(NCORES):
        b, h = divmod(core, 2)
        xb = np.asarray(x[b], np.float32)
        order = np.concatenate(
            [np.arange(QB * (2 * t + h), QB * (2 * t + h) + QB)
             for t in range(NSLOT)])
        xq = xb[order]
        xq4 = np.ascontiguousarray(
            xq.reshape(4, 256, 8, P).transpose(0, 3, 2, 1)
        ).astype(ml_dtypes.bfloat16)
        in_maps.append({
            "xq4": xq4,
            "wq2": wq2, "wk2": wk2, "wv2": wv2,
            "maskT": masks[h],
        })
    return in_maps


def run(inputs, trace=False):
    nc = _get_nc()
    in_maps = _prep_inputs(inputs["x"], inputs["Wq"], inputs["Wk"],
                           inputs["Wv"])
    res = bass_utils.run_bass_kernel_spmd(
        nc, in_maps, core_ids=list(range(NCORES)), trace=trace)
    out = np.empty((B, S, D), np.float32)
    for core in range(NCORES):
        b, h = divmod(core, 2)
        oc = np.asarray(res.results[core]["out"]).astype(np.float32)
        for t in range(NSLOT):
            out[b, QB * (2 * t + h):QB * (2 * t + h) + QB] = \
                oc[QB * t:QB * t + QB]
    return out, res


def kernel(**inputs):
    out, _ = run(inputs, trace=False)
    return out
